# revision 21
# baseline (speedup 1.0000x reference)
"""CRF input-energy kernel for Trainium2 (8 NeuronCores, data-parallel on batch).

Computes out[B,T,U] = X @ kernel + bias, with left/right boundary energies
added at t=0 and t=T-1.

Strategy: pure data parallel — each of the 8 cores gets 8 of the 64 batch
sequences. Host-side we relayout each core's X shard to d-major [D, R]
(R = 8*4096 rows) so the contraction dim D=128 lands on SBUF partitions with
fully contiguous DMA. The weight [128,32] stays stationary in the PE array
(replicated into all four 32-column groups via tile_position), and X streams
through as the 512-wide moving operand: each matmul produces a transposed
energy block [32u, 512r] in one of four PSUM partition groups. The
VectorEngine adds bias (pre-broadcast per-partition tables, with the t=0 /
t=T-1 boundary energies folded into the even/odd-tile variants) while copying
PSUM->SBUF. The blocked transposed output [128, R*U/128] is un-permuted on
host.
"""

import sys
import types

import numpy as np

import concourse.bass as bass
import concourse.tile as tile
from concourse import bacc, mybir
from concourse.bass import ds, ts
from concourse.bass_utils import run_bass_kernel_spmd


def _ensure_axon_hooks_importable():
    """bass_utils imports antenv.axon_hooks when tracing is requested (e.g. a
    stray BASS_TRACE=1 in the environment); some images lack that submodule.
    Register a functional stand-in so the import never hard-fails."""
    try:
        from antenv import axon_hooks  # noqa: F401
        return
    except ImportError:
        pass
    mod = types.ModuleType("antenv.axon_hooks")
    _hook = [None]
    mod.set_axon_ntff_profile_hook = lambda h: _hook.__setitem__(0, h)
    mod.get_axon_ntff_profile_hook = lambda: _hook[0]
    sys.modules["antenv.axon_hooks"] = mod
    import antenv

    antenv.axon_hooks = mod
    try:
        from trn_agent_boot.trn_boot import _ntff_profile_via_ctypes

        mod.set_axon_ntff_profile_hook(
            _ntff_profile_via_ctypes("/opt/axon/libaxon_pjrt.so")
        )
    except Exception:
        pass  # hook stays None -> bass_utils skips tracing gracefully


_ensure_axon_hooks_importable()

B, T, D, U = 64, 4096, 128, 32
N_CORES = 8
SEQ_PER_CORE = B // N_CORES      # 8
R = SEQ_PER_CORE * T             # 32768 rows per core
PB = 128                         # SBUF partition count
MOV = 512                        # moving-operand width (rows per matmul)
GRP = PB // U                    # 4 PE column groups / PSUM partition groups
ROWS_PER_PS = GRP * MOV          # 2048 rows per psum tile
NPS = R // ROWS_PER_PS           # 16 psum tiles per core
CH = 4096                        # X rows per compute chunk
NCH = R // CH                    # 8 chunks per core
PS_PER_CH = CH // ROWS_PER_PS    # 2 psum tiles per chunk
IN_SPLIT = 2                     # input DMAs per chunk (1 MB each)
OUT_COLS = R * U // PB           # 8192 output columns on device
CONST_COLS = U + 2 * MOV         # packed consts: w | btabl | btabr

_NC_CACHE = {}


def _build():
    nc = bacc.Bacc(
        "TRN2", target_bir_lowering=False, debug=False, num_devices=N_CORES
    )
    f32 = mybir.dt.float32
    xt = nc.dram_tensor("xt", [PB, R], f32, kind="ExternalInput").ap()
    cst = nc.dram_tensor("cst", [PB, CONST_COLS], f32, kind="ExternalInput").ap()
    out = nc.dram_tensor("out", [PB, OUT_COLS], f32, kind="ExternalOutput").ap()

    with tile.TileContext(nc) as tc:
        with (
            tc.tile_pool(name="consts", bufs=1) as consts,
            tc.tile_pool(name="xin", bufs=5) as xin,
            tc.tile_pool(name="outp", bufs=4) as outp,
            tc.tile_pool(name="ps", bufs=6, space=bass.MemorySpace.PSUM) as psp,
        ):
            cst_sb = consts.tile([PB, CONST_COLS], f32)
            nc.scalar.dma_start(cst_sb[:], cst[:])
            w_sb = cst_sb[:, 0:U]
            btabl_sb = cst_sb[:, U : U + MOV]
            btabr_sb = cst_sb[:, U + MOV : U + 2 * MOV]

            for n in range(NCH):
                xt_t = xin.tile([PB, CH], f32)
                if n == 0:
                    # small leading piece: its descriptor-gen is ~3x faster
                    # than a 2 MB transfer's, so the stream starts sooner
                    for lo, hw in ((0, 1024), (1024, 3072)):
                        nc.sync.dma_start(
                            xt_t[:, ds(lo, hw)], xt[:, ds(n * CH + lo, hw)]
                        )
                elif n == NCH - 1:
                    # split the final load (1M/1M/0.5M/0.5M) so the last psum
                    # tile's matmuls only wait on a 512 KB transfer
                    for lo, hw in ((0, 2048), (2048, 1024), (3072, 512), (3584, 512)):
                        nc.sync.dma_start(
                            xt_t[:, ds(lo, hw)], xt[:, ds(n * CH + lo, hw)]
                        )
                else:
                    nc.sync.dma_start(xt_t[:], xt[:, ds(n * CH, CH)])
                o_t = outp.tile([PB, PS_PER_CH * MOV], f32)
                for s in range(PS_PER_CH):
                    k = n * PS_PER_CH + s  # global psum-tile index
                    ps = psp.tile([PB, MOV], f32)
                    for g in range(GRP):
                        blk = s * GRP + g  # 512-row block within chunk
                        nc.tensor.matmul(
                            ps[g * U : (g + 1) * U, :],
                            w_sb[:],
                            xt_t[:, ds(blk * MOV, MOV)],
                            start=True,
                            stop=True,
                            tile_position=(0, g * U),
                        )
                    # psum tile = 2048 rows; a 4096-row sequence is exactly two
                    # tiles: even tiles hold the t=0 row at (partitions 0:32,
                    # col 0), odd tiles the t=T-1 row at (partitions 96:128,
                    # col 511). The bias tables carry the boundary energies.
                    tab = btabl_sb if k % 2 == 0 else btabr_sb
                    if k == NPS - 1:
                        # split the very last add by partition halves so the
                        # lower half (earlier rows) retires while the final
                        # 512-row piece is still streaming in
                        half = PB // 2
                        nc.vector.tensor_add(
                            o_t[0:half, ts(s, MOV)], ps[0:half, :], tab[0:half, :]
                        )
                        nc.vector.tensor_add(
                            o_t[half:PB, ts(s, MOV)],
                            ps[half:PB, :],
                            tab[half:PB, :],
                        )
                    else:
                        nc.vector.tensor_add(o_t[:, ts(s, MOV)], ps[:], tab[:])
                if n == NCH - 1:
                    # split the final store so the kernel tail only waits on
                    # the last psum tile's add, not the whole chunk; the very
                    # last store is further split by partition halves to chase
                    # the half-split add above
                    for s in range(PS_PER_CH):
                        if s == PS_PER_CH - 1:
                            half = PB // 2
                            nc.scalar.dma_start(
                                out[0:half, ds((n * PS_PER_CH + s) * MOV, MOV)],
                                o_t[0:half, ts(s, MOV)],
                            )
                            nc.scalar.dma_start(
                                out[half:PB, ds((n * PS_PER_CH + s) * MOV, MOV)],
                                o_t[half:PB, ts(s, MOV)],
                            )
                        else:
                            nc.scalar.dma_start(
                                out[:, ds((n * PS_PER_CH + s) * MOV, MOV)],
                                o_t[:, ts(s, MOV)],
                            )
                else:
                    nc.scalar.dma_start(
                        out[:, ds(n * PS_PER_CH * MOV, PS_PER_CH * MOV)], o_t[:]
                    )
    nc.compile()
    return nc


def _get_nc():
    if "nc" not in _NC_CACHE:
        _NC_CACHE["nc"] = _build()
    return _NC_CACHE["nc"]


def _make_in_maps(X, kern, bias, left_boundary, right_boundary):
    X = np.ascontiguousarray(np.asarray(X, dtype=np.float32))
    w = np.ascontiguousarray(np.asarray(kern, dtype=np.float32))
    bias = np.asarray(bias, dtype=np.float32)
    lb = np.asarray(left_boundary, dtype=np.float32)
    rb = np.asarray(right_boundary, dtype=np.float32)
    base = np.repeat(np.tile(bias, GRP)[:, None], MOV, axis=1)  # [128, 512]
    btabl = base.copy()
    btabl[0:U, 0] += lb
    btabr = base.copy()
    btabr[PB - U : PB, MOV - 1] += rb
    cst = np.concatenate([w, btabl, btabr], axis=1)  # [128, CONST_COLS]
    cst = np.ascontiguousarray(cst, dtype=np.float32)
    in_maps = []
    for c in range(N_CORES):
        Xc = X[c * SEQ_PER_CORE : (c + 1) * SEQ_PER_CORE].reshape(R, D)
        xt = np.ascontiguousarray(Xc.T)
        in_maps.append({"xt": xt, "cst": cst})
    return in_maps


def _unshard(results):
    outs = []
    for c in range(N_CORES):
        o = np.asarray(results[c]["out"])  # [128, OUT_COLS]
        # partition p = 32g + u ; column = 512k + c ; row = 2048k + 512g + c
        e = (
            o.reshape(GRP, U, NPS, MOV)
            .transpose(2, 0, 3, 1)
            .reshape(SEQ_PER_CORE, T, U)
        )
        outs.append(e)
    return np.concatenate(outs, axis=0)


def _run(inputs, trace=False):
    nc = _get_nc()
    in_maps = _make_in_maps(
        inputs["X"],
        inputs["kernel"],
        inputs["bias"],
        inputs["left_boundary"],
        inputs["right_boundary"],
    )
    last_err = None
    for attempt in range(3):
        try:
            res = run_bass_kernel_spmd(
                nc, in_maps, list(range(N_CORES)), trace=trace
            )
            return _unshard(res.results), res
        except Exception as e:  # transient device wedges (NRT_*) self-heal
            last_err = e
    raise last_err


def kernel(X, kernel, bias, left_boundary, right_boundary):
    out, _ = _run(
        {
            "X": X,
            "kernel": kernel,
            "bias": bias,
            "left_boundary": left_boundary,
            "right_boundary": right_boundary,
        }
    )
    return out


# revision 24
# speedup vs baseline: 1.0426x; 1.0426x over previous
"""CRF input-energy kernel for Trainium2 (8 NeuronCores, data-parallel on batch).

Computes out[B,T,U] = X @ kernel + bias, with left/right boundary energies
added at t=0 and t=T-1.

Strategy: pure data parallel — each of the 8 cores gets 8 of the 64 batch
sequences. Host-side we relayout each core's X shard to d-major [D, R]
(R = 8*4096 rows) so the contraction dim D=128 lands on SBUF partitions with
fully contiguous DMA. The weight [128,32] stays stationary in the PE array
(replicated into all four 32-column groups via tile_position), and X streams
through as the 512-wide moving operand: each matmul produces a transposed
energy block [32u, 512r] in one of four PSUM partition groups. The
VectorEngine adds bias (pre-broadcast per-partition tables, with the t=0 /
t=T-1 boundary energies folded into the even/odd-tile variants) while copying
PSUM->SBUF. The blocked transposed output [128, R*U/128] is un-permuted on
host.
"""

import sys
import types

import numpy as np

import concourse.bass as bass
import concourse.tile as tile
from concourse import bacc, mybir
from concourse.bass import ds, ts
from concourse.bass_utils import run_bass_kernel_spmd


def _ensure_axon_hooks_importable():
    """bass_utils imports antenv.axon_hooks when tracing is requested (e.g. a
    stray BASS_TRACE=1 in the environment); some images lack that submodule.
    Register a functional stand-in so the import never hard-fails."""
    try:
        from antenv import axon_hooks  # noqa: F401
        return
    except ImportError:
        pass
    mod = types.ModuleType("antenv.axon_hooks")
    _hook = [None]
    mod.set_axon_ntff_profile_hook = lambda h: _hook.__setitem__(0, h)
    mod.get_axon_ntff_profile_hook = lambda: _hook[0]
    sys.modules["antenv.axon_hooks"] = mod
    import antenv

    antenv.axon_hooks = mod
    try:
        from trn_agent_boot.trn_boot import _ntff_profile_via_ctypes

        mod.set_axon_ntff_profile_hook(
            _ntff_profile_via_ctypes("/opt/axon/libaxon_pjrt.so")
        )
    except Exception:
        pass  # hook stays None -> bass_utils skips tracing gracefully


_ensure_axon_hooks_importable()

B, T, D, U = 64, 4096, 128, 32
N_CORES = 8
SEQ_PER_CORE = B // N_CORES      # 8
R = SEQ_PER_CORE * T             # 32768 rows per core
PB = 128                         # SBUF partition count
MOV = 512                        # moving-operand width (rows per matmul)
GRP = PB // U                    # 4 PE column groups / PSUM partition groups
ROWS_PER_PS = GRP * MOV          # 2048 rows per psum tile
NPS = R // ROWS_PER_PS           # 16 psum tiles per core
CH = 4096                        # X rows per compute chunk
NCH = R // CH                    # 8 chunks per core
PS_PER_CH = CH // ROWS_PER_PS    # 2 psum tiles per chunk
IN_SPLIT = 2                     # input DMAs per chunk (1 MB each)
OUT_COLS = R * U // PB           # 8192 output columns on device
CONST_COLS = U + 2 * MOV         # packed consts: w | btabl | btabr

_NC_CACHE = {}


def _build():
    nc = bacc.Bacc(
        "TRN2", target_bir_lowering=False, debug=False, num_devices=N_CORES
    )
    f32 = mybir.dt.float32
    xt = nc.dram_tensor("xt", [PB, R], f32, kind="ExternalInput").ap()
    cst = nc.dram_tensor("cst", [PB, CONST_COLS], f32, kind="ExternalInput").ap()
    out = nc.dram_tensor("out", [PB, OUT_COLS], f32, kind="ExternalOutput").ap()

    with tile.TileContext(nc) as tc:
        with (
            tc.tile_pool(name="consts", bufs=1) as consts,
            tc.tile_pool(name="xin", bufs=5) as xin,
            tc.tile_pool(name="outp", bufs=4) as outp,
            tc.tile_pool(name="ps", bufs=6, space=bass.MemorySpace.PSUM) as psp,
        ):
            cst_sb = consts.tile([PB, CONST_COLS], f32)
            nc.scalar.dma_start(cst_sb[:], cst[:])
            w_sb = cst_sb[:, 0:U]
            btabl_sb = cst_sb[:, U : U + MOV]
            btabr_sb = cst_sb[:, U + MOV : U + 2 * MOV]

            for n in range(NCH):
                xt_t = xin.tile([PB, CH], f32)
                if n == NCH - 1:
                    # split the final load (1M/1M/0.5M/0.5M) so the last psum
                    # tile's matmuls only wait on a 512 KB transfer
                    for lo, hw in ((0, 2048), (2048, 1024), (3072, 512), (3584, 512)):
                        nc.sync.dma_start(
                            xt_t[:, ds(lo, hw)], xt[:, ds(n * CH + lo, hw)]
                        )
                else:
                    nc.sync.dma_start(xt_t[:], xt[:, ds(n * CH, CH)])
                o_t = outp.tile([PB, PS_PER_CH * MOV], f32)
                for s in range(PS_PER_CH):
                    k = n * PS_PER_CH + s  # global psum-tile index
                    ps = psp.tile([PB, MOV], f32)
                    for g in range(GRP):
                        blk = s * GRP + g  # 512-row block within chunk
                        nc.tensor.matmul(
                            ps[g * U : (g + 1) * U, :],
                            w_sb[:],
                            xt_t[:, ds(blk * MOV, MOV)],
                            start=True,
                            stop=True,
                            tile_position=(0, g * U),
                        )
                    # psum tile = 2048 rows; a 4096-row sequence is exactly two
                    # tiles: even tiles hold the t=0 row at (partitions 0:32,
                    # col 0), odd tiles the t=T-1 row at (partitions 96:128,
                    # col 511). The bias tables carry the boundary energies.
                    tab = btabl_sb if k % 2 == 0 else btabr_sb
                    nc.vector.tensor_add(o_t[:, ts(s, MOV)], ps[:], tab[:])
                if n == NCH - 1:
                    # split the final store so the kernel tail only waits on
                    # the last psum tile's add, not the whole chunk
                    for s in range(PS_PER_CH):
                        nc.scalar.dma_start(
                            out[:, ds((n * PS_PER_CH + s) * MOV, MOV)],
                            o_t[:, ts(s, MOV)],
                        )
                else:
                    nc.scalar.dma_start(
                        out[:, ds(n * PS_PER_CH * MOV, PS_PER_CH * MOV)], o_t[:]
                    )
    nc.compile()
    return nc


def _get_nc():
    if "nc" not in _NC_CACHE:
        _NC_CACHE["nc"] = _build()
    return _NC_CACHE["nc"]


def _make_in_maps(X, kern, bias, left_boundary, right_boundary):
    X = np.ascontiguousarray(np.asarray(X, dtype=np.float32))
    w = np.ascontiguousarray(np.asarray(kern, dtype=np.float32))
    bias = np.asarray(bias, dtype=np.float32)
    lb = np.asarray(left_boundary, dtype=np.float32)
    rb = np.asarray(right_boundary, dtype=np.float32)
    base = np.repeat(np.tile(bias, GRP)[:, None], MOV, axis=1)  # [128, 512]
    btabl = base.copy()
    btabl[0:U, 0] += lb
    btabr = base.copy()
    btabr[PB - U : PB, MOV - 1] += rb
    cst = np.concatenate([w, btabl, btabr], axis=1)  # [128, CONST_COLS]
    cst = np.ascontiguousarray(cst, dtype=np.float32)
    in_maps = []
    for c in range(N_CORES):
        Xc = X[c * SEQ_PER_CORE : (c + 1) * SEQ_PER_CORE].reshape(R, D)
        xt = np.ascontiguousarray(Xc.T)
        in_maps.append({"xt": xt, "cst": cst})
    return in_maps


def _unshard(results):
    outs = []
    for c in range(N_CORES):
        o = np.asarray(results[c]["out"])  # [128, OUT_COLS]
        # partition p = 32g + u ; column = 512k + c ; row = 2048k + 512g + c
        e = (
            o.reshape(GRP, U, NPS, MOV)
            .transpose(2, 0, 3, 1)
            .reshape(SEQ_PER_CORE, T, U)
        )
        outs.append(e)
    return np.concatenate(outs, axis=0)


def _run(inputs, trace=False):
    nc = _get_nc()
    in_maps = _make_in_maps(
        inputs["X"],
        inputs["kernel"],
        inputs["bias"],
        inputs["left_boundary"],
        inputs["right_boundary"],
    )
    last_err = None
    for attempt in range(3):
        try:
            res = run_bass_kernel_spmd(
                nc, in_maps, list(range(N_CORES)), trace=trace
            )
            return _unshard(res.results), res
        except Exception as e:  # transient device wedges (NRT_*) self-heal
            last_err = e
    raise last_err


def kernel(X, kernel, bias, left_boundary, right_boundary):
    out, _ = _run(
        {
            "X": X,
            "kernel": kernel,
            "bias": bias,
            "left_boundary": left_boundary,
            "right_boundary": right_boundary,
        }
    )
    return out


# revision 25
# speedup vs baseline: 1.1194x; 1.0736x over previous
"""CRF input-energy kernel for Trainium2 (8 NeuronCores, data-parallel on batch).

Computes out[B,T,U] = X @ kernel + bias, with left/right boundary energies
added at t=0 and t=T-1.

Strategy: pure data parallel — each of the 8 cores gets 8 of the 64 batch
sequences. Host-side we relayout each core's X shard to d-major [D, R]
(R = 8*4096 rows) so the contraction dim D=128 lands on SBUF partitions with
fully contiguous DMA. The weight [128,32] stays stationary in the PE array
(replicated into all four 32-column groups via tile_position), and X streams
through as the 512-wide moving operand: each matmul produces a transposed
energy block [32u, 512r] in one of four PSUM partition groups. The
VectorEngine adds bias (pre-broadcast per-partition tables, with the t=0 /
t=T-1 boundary energies folded into the even/odd-tile variants) while copying
PSUM->SBUF. The blocked transposed output [128, R*U/128] is un-permuted on
host.
"""

import sys
import types

import numpy as np

import concourse.bass as bass
import concourse.tile as tile
from concourse import bacc, mybir
from concourse.bass import ds, ts
from concourse.bass_utils import run_bass_kernel_spmd


def _ensure_axon_hooks_importable():
    """bass_utils imports antenv.axon_hooks when tracing is requested (e.g. a
    stray BASS_TRACE=1 in the environment); some images lack that submodule.
    Register a functional stand-in so the import never hard-fails."""
    try:
        from antenv import axon_hooks  # noqa: F401
        return
    except ImportError:
        pass
    mod = types.ModuleType("antenv.axon_hooks")
    _hook = [None]
    mod.set_axon_ntff_profile_hook = lambda h: _hook.__setitem__(0, h)
    mod.get_axon_ntff_profile_hook = lambda: _hook[0]
    sys.modules["antenv.axon_hooks"] = mod
    import antenv

    antenv.axon_hooks = mod
    try:
        from trn_agent_boot.trn_boot import _ntff_profile_via_ctypes

        mod.set_axon_ntff_profile_hook(
            _ntff_profile_via_ctypes("/opt/axon/libaxon_pjrt.so")
        )
    except Exception:
        pass  # hook stays None -> bass_utils skips tracing gracefully


_ensure_axon_hooks_importable()

B, T, D, U = 64, 4096, 128, 32
N_CORES = 8
SEQ_PER_CORE = B // N_CORES      # 8
R = SEQ_PER_CORE * T             # 32768 rows per core
PB = 128                         # SBUF partition count
MOV = 512                        # moving-operand width (rows per matmul)
GRP = PB // U                    # 4 PE column groups / PSUM partition groups
ROWS_PER_PS = GRP * MOV          # 2048 rows per psum tile
NPS = R // ROWS_PER_PS           # 16 psum tiles per core
CH = 4096                        # X rows per compute chunk
NCH = R // CH                    # 8 chunks per core
PS_PER_CH = CH // ROWS_PER_PS    # 2 psum tiles per chunk
IN_SPLIT = 2                     # input DMAs per chunk (1 MB each)
OUT_COLS = R * U // PB           # 8192 output columns on device
CONST_COLS = U + 2 * MOV         # packed consts: w | btabl | btabr

_NC_CACHE = {}


def _build():
    nc = bacc.Bacc(
        "TRN2", target_bir_lowering=False, debug=False, num_devices=N_CORES
    )
    f32 = mybir.dt.float32
    xt = nc.dram_tensor("xt", [PB, R], f32, kind="ExternalInput").ap()
    cst = nc.dram_tensor("cst", [PB, CONST_COLS], f32, kind="ExternalInput").ap()
    out = nc.dram_tensor("out", [PB, OUT_COLS], f32, kind="ExternalOutput").ap()

    with tile.TileContext(nc) as tc:
        with (
            tc.tile_pool(name="consts", bufs=1) as consts,
            tc.tile_pool(name="xin", bufs=6) as xin,
            tc.tile_pool(name="outp", bufs=4) as outp,
            tc.tile_pool(name="ps", bufs=8, space=bass.MemorySpace.PSUM) as psp,
        ):
            cst_sb = consts.tile([PB, CONST_COLS], f32)
            nc.scalar.dma_start(cst_sb[:], cst[:])
            w_sb = cst_sb[:, 0:U]
            btabl_sb = cst_sb[:, U : U + MOV]
            btabr_sb = cst_sb[:, U + MOV : U + 2 * MOV]

            for n in range(NCH):
                xt_t = xin.tile([PB, CH], f32)
                if n == NCH - 1:
                    # split the final load (1M/1M/0.5M/0.5M) so the last psum
                    # tile's matmuls only wait on a 512 KB transfer
                    for lo, hw in ((0, 2048), (2048, 1024), (3072, 512), (3584, 512)):
                        nc.sync.dma_start(
                            xt_t[:, ds(lo, hw)], xt[:, ds(n * CH + lo, hw)]
                        )
                else:
                    nc.sync.dma_start(xt_t[:], xt[:, ds(n * CH, CH)])
                o_t = outp.tile([PB, PS_PER_CH * MOV], f32)
                for s in range(PS_PER_CH):
                    k = n * PS_PER_CH + s  # global psum-tile index
                    ps = psp.tile([PB, MOV], f32)
                    for g in range(GRP):
                        blk = s * GRP + g  # 512-row block within chunk
                        nc.tensor.matmul(
                            ps[g * U : (g + 1) * U, :],
                            w_sb[:],
                            xt_t[:, ds(blk * MOV, MOV)],
                            start=True,
                            stop=True,
                            tile_position=(0, g * U),
                        )
                    # psum tile = 2048 rows; a 4096-row sequence is exactly two
                    # tiles: even tiles hold the t=0 row at (partitions 0:32,
                    # col 0), odd tiles the t=T-1 row at (partitions 96:128,
                    # col 511). The bias tables carry the boundary energies.
                    tab = btabl_sb if k % 2 == 0 else btabr_sb
                    nc.vector.tensor_add(o_t[:, ts(s, MOV)], ps[:], tab[:])
                if n == NCH - 1:
                    # split the final store so the kernel tail only waits on
                    # the last psum tile's add, not the whole chunk
                    for s in range(PS_PER_CH):
                        nc.scalar.dma_start(
                            out[:, ds((n * PS_PER_CH + s) * MOV, MOV)],
                            o_t[:, ts(s, MOV)],
                        )
                else:
                    nc.scalar.dma_start(
                        out[:, ds(n * PS_PER_CH * MOV, PS_PER_CH * MOV)], o_t[:]
                    )
    nc.compile()
    return nc


def _get_nc():
    if "nc" not in _NC_CACHE:
        _NC_CACHE["nc"] = _build()
    return _NC_CACHE["nc"]


def _make_in_maps(X, kern, bias, left_boundary, right_boundary):
    X = np.ascontiguousarray(np.asarray(X, dtype=np.float32))
    w = np.ascontiguousarray(np.asarray(kern, dtype=np.float32))
    bias = np.asarray(bias, dtype=np.float32)
    lb = np.asarray(left_boundary, dtype=np.float32)
    rb = np.asarray(right_boundary, dtype=np.float32)
    base = np.repeat(np.tile(bias, GRP)[:, None], MOV, axis=1)  # [128, 512]
    btabl = base.copy()
    btabl[0:U, 0] += lb
    btabr = base.copy()
    btabr[PB - U : PB, MOV - 1] += rb
    cst = np.concatenate([w, btabl, btabr], axis=1)  # [128, CONST_COLS]
    cst = np.ascontiguousarray(cst, dtype=np.float32)
    in_maps = []
    for c in range(N_CORES):
        Xc = X[c * SEQ_PER_CORE : (c + 1) * SEQ_PER_CORE].reshape(R, D)
        xt = np.ascontiguousarray(Xc.T)
        in_maps.append({"xt": xt, "cst": cst})
    return in_maps


def _unshard(results):
    outs = []
    for c in range(N_CORES):
        o = np.asarray(results[c]["out"])  # [128, OUT_COLS]
        # partition p = 32g + u ; column = 512k + c ; row = 2048k + 512g + c
        e = (
            o.reshape(GRP, U, NPS, MOV)
            .transpose(2, 0, 3, 1)
            .reshape(SEQ_PER_CORE, T, U)
        )
        outs.append(e)
    return np.concatenate(outs, axis=0)


def _run(inputs, trace=False):
    nc = _get_nc()
    in_maps = _make_in_maps(
        inputs["X"],
        inputs["kernel"],
        inputs["bias"],
        inputs["left_boundary"],
        inputs["right_boundary"],
    )
    last_err = None
    for attempt in range(3):
        try:
            res = run_bass_kernel_spmd(
                nc, in_maps, list(range(N_CORES)), trace=trace
            )
            return _unshard(res.results), res
        except Exception as e:  # transient device wedges (NRT_*) self-heal
            last_err = e
    raise last_err


def kernel(X, kernel, bias, left_boundary, right_boundary):
    out, _ = _run(
        {
            "X": X,
            "kernel": kernel,
            "bias": bias,
            "left_boundary": left_boundary,
            "right_boundary": right_boundary,
        }
    )
    return out
